# revision 27
# baseline (speedup 1.0000x reference)
"""Trainium2 Bass kernel for nn_Jitter: block-wise bilinear jitter (grid_sample).

Math (per sample s, 16x16 block (by,bx), PROB=1.0, align_corners=True):
  dx = 511*rx - 255.5, dy = 511*ry - 255.5   (rx,ry = random_flow_lr in [0,1))
  out[c, 16by+ii, 16bx+jj] = bilinear(x[c], y=16by+ii+dy, x=16bx+jj+dx), zero pad.
Since floor(j+dx) = j+floor(dx), each block needs a 17x17 source window at
integer offset (floor(dy), floor(dx)) and constant fractional weights (wy, wx).

Design (pure data parallel, 4 samples/core on 8 cores):
  - Host stages x as fp16 panels with the 3 channels interleaved per row:
    panel t covers cols [8t-17, 8t+9) (stride 8, 26 cols/section, 78/row),
    rows -17..529, zero-padded on every edge. With coordinates clamped to
    c0 in [-17,512], r0 in [-17,513], every out-of-bounds tap reads a real
    zero -- no validity masks anywhere. Host also precomputes the per-block
    gather index and the 4 blend weights (tiny [128,32] arrays).
  - 32 indirect DMAs (one per bx; idx [P,1] is the only HW-supported form)
    each gather 128 windows of 17x78 fp16 (all 3 channels, one contiguous
    run) into a single resident win tile; SWDGE costs ~1.2us/instruction.
  - Blend per bx with per-partition scalar weights; partition p = (s,by):
      ScalarE: tmp1 = win[1:17]*wy; tmp2 = win[0:16]*(1-wy) (even k)
      Vector:  tmp2 (odd k, 4x tensor_scalar), s = tmp1+tmp2 (2x),
               a = s[..0:16]*(1-wx) (4x),
               out_f32 = s[..1:17]*wx + a (scalar_tensor_tensor, writes outt)
  - Output: per (group, channel, sample) HWDGE writes y[s,c,:,cols]; that AP
    merges to [512,128] whose outer dim sprays descriptors over all 16 SDMA
    engines (a [4,...]-outer AP runs 3x slower). Last group split over the
    Sync and Scalar queues to shorten the tail.
"""

import numpy as np

import concourse.bacc as bacc
import concourse.bass as bass
import concourse.mybir as mybir
import concourse.tile as tile
from concourse.bass import IndirectOffsetOnAxis
from concourse.bass_utils import run_bass_kernel_spmd

F32 = mybir.dt.float32
F16 = mybir.dt.float16
I32 = mybir.dt.int32

B, C, H, W = 32, 3, 512, 512
NCORES = 8
S = B // NCORES            # 4 samples per core
NBY, NBX = H // 16, W // 16
P = S * NBY                # 128 partitions = (s, by)
STR = 4                    # panel column stride
SEC = STR + 18             # 22 cols per channel section
ROWW = C * SEC             # 66 elems per panel row
NPAN = (512 + 17) // STR + 1   # 133 panels
PR = 17 + H + 18           # 547 rows per panel
PANEL = PR * ROWW          # 42666
SPLANE = NPAN * PANEL      # 2858622 per sample
FPAD = 2048
XSN = FPAD + S * SPLANE + FPAD
ELEM = 17 * ROWW           # 1326 fp16 gathered per window
KC = 8                     # bx per group
NG = NBX // KC             # 4 groups

_CACHE = {}


def _stage_panels(xs_core):
    """xs_core: [S,3,512,512] -> staged fp16 panel buffer [XSN]."""
    x16 = np.ascontiguousarray(xs_core).astype(np.float16)
    xpad = np.zeros((S, C, PR, 17 + W + SEC), dtype=np.float16)
    xpad[:, :, 17:17 + H, 17:17 + W] = x16
    out = np.zeros(XSN, dtype=np.float16)
    body = out[FPAD:FPAD + S * SPLANE].reshape(S, NPAN, PR, C, SEC)
    for t in range(NPAN):
        body[:, t] = xpad[:, :, :, STR * t:STR * t + SEC].transpose(0, 2, 1, 3)
    return out


def _coords(rfl):
    """rfl: [S,2,32,32] -> idx [P,NBX] i32, weights [P, 4*NBX] f32
    (wya | wyb | wxa | wxb). Partition p = s*NBY + by."""
    rx = rfl[:, 0].astype(np.float32)      # [s, by, bx]
    ry = rfl[:, 1].astype(np.float32)
    vx = np.float32(511.0) * rx + np.float32(0.5)
    vy = np.float32(511.0) * ry + np.float32(0.5)
    flx = np.floor(vx)
    fly = np.floor(vy)
    wx = vx - flx
    wy = vy - fly
    bx = np.arange(NBX, dtype=np.float32)[None, None, :]
    by = np.arange(NBY, dtype=np.float32)[None, :, None]
    c0 = np.clip(flx + 16.0 * bx - 256.0, -17.0, 512.0)
    r0 = np.clip(fly + 16.0 * by - 256.0, -17.0, 513.0)
    u = c0 + 17.0
    t = np.floor(u / STR)
    o = u - STR * t
    s = np.arange(S, dtype=np.float64)[:, None, None]
    idx = (FPAD + s * SPLANE + t.astype(np.float64) * PANEL
           + (r0 + 17.0).astype(np.float64) * ROWW + o).astype(np.int32)
    wts = np.concatenate([1.0 - wy, wy, 1.0 - wx, wx],
                         axis=2).astype(np.float32)
    return idx.reshape(P, NBX), wts.reshape(P, 4 * NBX)


def _build_nc():
    nc = bacc.Bacc("TRN2", target_bir_lowering=False, debug=False,
                   num_devices=NCORES)

    xs = nc.dram_tensor("xs", [XSN, 1], F16, kind="ExternalInput")
    idx = nc.dram_tensor("idx", [P, NBX], I32, kind="ExternalInput")
    wts = nc.dram_tensor("wts", [P, 4 * NBX], F32, kind="ExternalInput")
    y = nc.dram_tensor("y", [S, C, H, W], F32, kind="ExternalOutput")

    with tile.TileContext(nc) as tc:
        with (
            tc.tile_pool(name="prep", bufs=1) as pp,
            tc.tile_pool(name="blend", bufs=3) as lp,
            tc.tile_pool(name="out", bufs=2) as op,
        ):
            v = nc.vector
            A = mybir.AluOpType
            Copy = mybir.ActivationFunctionType.Copy

            idxi = pp.tile([P, NBX], I32, tag="idxi")
            nc.sync.dma_start(idxi[:], idx[:])
            wt = pp.tile([P, 4 * NBX], F32, tag="wt")
            nc.sync.dma_start(wt[:], wts[:])
            wya = wt[:][:, 0:NBX]
            wyb = wt[:][:, NBX:2 * NBX]
            wxa = wt[:][:, 2 * NBX:3 * NBX]
            wxb = wt[:][:, 3 * NBX:4 * NBX]

            win = pp.tile([P, NBX, ELEM], F16, tag="win")
            for bx in range(NBX):
                nc.gpsimd.indirect_dma_start(
                    out=win[:, bx, :], out_offset=None,
                    in_=xs[:],
                    in_offset=IndirectOffsetOnAxis(
                        ap=idxi[:][:, bx:bx + 1], axis=0),
                )

            yv = y[:]
            for g in range(NG):
                outt = op.tile([P, C, 16, KC * 16], F32, tag="outt")
                for k in range(KC):
                    bx = g * KC + k
                    w4 = win[:][:, bx].rearrange("p (ii c w) -> p c ii w",
                                                 c=C, w=SEC)
                    tmp1 = lp.tile([P, C, 16, 18], F16, tag="tmp1")
                    tmp2 = lp.tile([P, C, 16, 18], F16, tag="tmp2")
                    late = bx >= 24   # gathers done; GpSimd idle, free Scalar
                    if late:
                        nc.gpsimd.tensor_scalar(tmp1[:], w4[:, :, 1:17, 0:18],
                                                wyb[:, bx:bx + 1], None,
                                                A.mult)
                    else:
                        nc.scalar.activation(tmp1[:], w4[:, :, 1:17, 0:18],
                                             Copy, scale=wyb[:, bx:bx + 1])
                    if late or k % 3 == 1:
                        v.tensor_scalar(tmp2[:], w4[:, :, 0:16, 0:18],
                                        wya[:, bx:bx + 1], None, A.mult)
                    else:
                        nc.scalar.activation(tmp2[:], w4[:, :, 0:16, 0:18],
                                             Copy, scale=wya[:, bx:bx + 1])
                    s = lp.tile([P, C, 16, 18], F16, tag="s")
                    v.tensor_tensor(s[:], tmp1[:], tmp2[:], A.add)
                    av = lp.tile([P, C, 16, 16], F16, tag="av")
                    v.tensor_scalar(av[:], s[:, :, :, 0:16],
                                    wxa[:, bx:bx + 1], None, A.mult)
                    v.scalar_tensor_tensor(
                        outt[:, :, :, k * 16:(k + 1) * 16],
                        s[:, :, :, 1:17], wxb[:, bx:bx + 1], av[:],
                        A.mult, A.add)
                for c in range(C):
                    for sm in range(S):
                        # y[s,c,:,cols] merges to [512,128]: outer dim 512
                        # sprays descriptors across all 16 SDMA engines.
                        eng = (nc.scalar if g == NG - 1 and (c + sm) % 2
                               else nc.sync)
                        eng.dma_start(
                            out=yv[sm, c, :,
                                   g * KC * 16:(g + 1) * KC * 16],
                            in_=outt[sm * NBY:(sm + 1) * NBY, c, :, :])

    nc.compile()
    return nc


def get_nc():
    if "nc" not in _CACHE:
        _CACHE["nc"] = _build_nc()
    return _CACHE["nc"]


def make_in_maps(x, random_flow_lr):
    x = np.ascontiguousarray(x, dtype=np.float32)
    rfl = np.ascontiguousarray(random_flow_lr, dtype=np.float32)
    in_maps = []
    for k in range(NCORES):
        xsb = _stage_panels(x[k * S:(k + 1) * S]).reshape(XSN, 1)
        idxv, wtsv = _coords(rfl[k * S:(k + 1) * S])
        in_maps.append({"xs": xsb, "idx": idxv, "wts": wtsv})
    return in_maps


def kernel(x, random_flow_lr):
    nc = get_nc()
    in_maps = make_in_maps(x, random_flow_lr)
    res = run_bass_kernel_spmd(nc, in_maps, core_ids=list(range(NCORES)))
    return np.concatenate([r["y"] for r in res.results], axis=0)
